# revision 1
# baseline (speedup 1.0000x reference)
"""Conformer layer kernel for 8 Trainium2 NeuronCores.

Strategy: data-parallel over tokens for the FFN blocks (the dominant
matmuls), which are pointwise over (T, B).  Each core gets 1/8 of the
T*B tokens of the LayerNormed activations plus the FFN weights, and
computes silu(x @ w1.T + b1) @ w2.T + b2 on-device via a Bass/Tile
kernel (channel-major layout: tokens on the matmul free dim, feature
dim on partitions/contraction).  The remaining glue (LayerNorms, the
tiny B=16-sequence attention, depthwise conv + GroupNorm) runs on
host.  Falls back to a host FFN if device execution fails.
"""

import numpy as np

T, B, D, H, F, K, MRP = 2048, 16, 256, 4, 1024, 31, 2
DH = D // H
N_CORES = 8

# ----------------------------------------------------------------- host math


def _ln(x, g, b, eps=1e-5):
    m = x.mean(-1, keepdims=True)
    v = x.var(-1, keepdims=True)
    return (x - m) / np.sqrt(v + eps) * g + b


def _silu(x):
    return x / (1.0 + np.exp(-x))


def _sigmoid(x):
    return 1.0 / (1.0 + np.exp(-x))


def _rel_emb(table, lq, lk):
    idx = np.clip(np.arange(lk)[None, :] - np.arange(lq)[:, None], -MRP, MRP) + MRP
    return table[idx]  # (lq, lk, DH)


def _softmax(x, axis=-1):
    m = x.max(axis=axis, keepdims=True)
    e = np.exp(x - m)
    return e / e.sum(axis=axis, keepdims=True)


def _attention(x, wq, bq, wk, bk, wv, bv, wo, bo, rel_k, rel_v):
    q = x @ wq.T + bq
    k = x @ wk.T + bk
    v = x @ wv.T + bv
    qh = q.reshape(T, B, H, DH)
    kh = k.reshape(T, B, H, DH)
    vh = v.reshape(T, B, H, DH)
    a1 = np.einsum("tqhd,tkhd->thqk", qh, kh, optimize=True)
    a2 = np.einsum("tqhd,qkd->thqk", qh, _rel_emb(rel_k, B, B), optimize=True)
    p = _softmax((a1 + a2) / np.sqrt(np.float32(DH)), axis=-1)
    w1 = np.einsum("thqk,tkhd->thqd", p, vh, optimize=True)
    w2 = np.einsum("thqk,qkd->thqd", p, _rel_emb(rel_v, B, B), optimize=True)
    o = (w1 + w2).transpose(0, 2, 1, 3).reshape(T, B, D)
    return o @ wo.T + bo


def _conv_module(x, ln_g, ln_b, pw1_w, pw1_b, dw_w, dw_b, gn_g, gn_b, pw2_w, pw2_b):
    y = _ln(x.transpose(1, 0, 2), ln_g, ln_b)  # (B,T,D)
    y = y.transpose(0, 2, 1)  # (B,D,T)
    y = np.einsum("oc,bct->bot", pw1_w, y, optimize=True) + pw1_b[:, None]
    y = y[:, :D] * _sigmoid(y[:, D:])  # GLU over channels
    pad = (K - 1) // 2
    yp = np.pad(y, ((0, 0), (0, 0), (pad, pad)))
    out = np.zeros_like(y)
    w = dw_w[:, 0, :]  # (D, K)
    for k in range(K):
        out += yp[:, :, k : k + T] * w[None, :, k : k + 1]
    y = out + dw_b[None, :, None]
    m = y.mean((1, 2), keepdims=True)
    v = y.var((1, 2), keepdims=True)
    y = (y - m) / np.sqrt(v + 1e-5) * gn_g[:, None] + gn_b[:, None]
    y = _silu(y)
    y = np.einsum("oc,bct->bot", pw2_w, y, optimize=True) + pw2_b[:, None]
    return y.transpose(2, 0, 1)  # (T,B,D)


# ------------------------------------------------------------- device FFN

_FFN_CACHE = {}


def _build_ffn_bass():
    """Bass/Tile SPMD kernel: out.T = w2 @ silu(w1 @ x.T + b1) + b2 for a
    shard of NTOK tokens, channel-major ([feature, token] layout)."""
    import concourse.bass as bass
    import concourse.mybir as mybir
    import concourse.tile as tile
    from concourse import tile_patch_shim  # noqa: F401  (installed below)

    NTOK = (T * B) // N_CORES  # 4096 tokens per core
    CH = 512  # token chunk (matmul free dim)
    nc = bass.Bass("TRN2", target_bir_lowering=False, debug=False,
                   num_devices=N_CORES)
    xT = nc.declare_dram_parameter("xT", [D, NTOK], mybir.dt.float32,
                                   isOutput=False)
    w1T = nc.declare_dram_parameter("w1T", [D, F], mybir.dt.float32,
                                    isOutput=False)
    b1c = nc.declare_dram_parameter("b1c", [F, 1], mybir.dt.float32,
                                    isOutput=False)
    w2T = nc.declare_dram_parameter("w2T", [F, D], mybir.dt.float32,
                                    isOutput=False)
    b2c = nc.declare_dram_parameter("b2c", [D, 1], mybir.dt.float32,
                                    isOutput=False)
    outT = nc.declare_dram_parameter("outT", [D, NTOK], mybir.dt.float32,
                                     isOutput=True)

    KD, KF, MD = D // 128, F // 128, D // 128  # 2, 8, 2
    NCH = NTOK // CH  # 8 chunks

    with tile.TileContext(nc) as tc:
        with (
            tc.tile_pool(name="wpool", bufs=1) as wpool,
            tc.tile_pool(name="xpool", bufs=3) as xpool,
            tc.tile_pool(name="hpool", bufs=3) as hpool,
            tc.tile_pool(name="opool", bufs=3) as opool,
            tc.tile_pool(name="ps", bufs=4, space="PSUM") as ps,
        ):
            # Tiles are [partition<=128, free...]; K-tiles live on free dims.
            w1t = wpool.tile([128, KD, F], mybir.dt.float32, tag="w1")
            w2t = wpool.tile([128, KF, D], mybir.dt.float32, tag="w2")
            b1t = wpool.tile([128, KF, 1], mybir.dt.float32, tag="b1")
            b2t = wpool.tile([128, MD, 1], mybir.dt.float32, tag="b2")
            xTr = xT.rearrange("(kd p) n -> kd p n", p=128)
            w1r = w1T.rearrange("(kd p) f -> kd p f", p=128)
            w2r = w2T.rearrange("(kf p) d -> kf p d", p=128)
            b1r = b1c.rearrange("(kf p) o -> kf p o", p=128)
            b2r = b2c.rearrange("(md p) o -> md p o", p=128)
            outr = outT.rearrange("(md p) n -> md p n", p=128)
            for kd in range(KD):
                nc.sync.dma_start(out=w1t[:, kd], in_=w1r[kd])
            for kf in range(KF):
                nc.sync.dma_start(out=w2t[:, kf], in_=w2r[kf])
                nc.sync.dma_start(out=b1t[:, kf], in_=b1r[kf])
            for md in range(MD):
                nc.sync.dma_start(out=b2t[:, md], in_=b2r[md])

            for ci in range(NCH):
                sl = bass.ts(ci, CH)
                xt = xpool.tile([128, KD, CH], mybir.dt.float32, tag="x")
                for kd in range(KD):
                    nc.sync.dma_start(out=xt[:, kd], in_=xTr[kd, :, sl])
                ht = hpool.tile([128, KF, CH], mybir.dt.float32, tag="h")
                for mf in range(KF):
                    pt = ps.tile([128, CH], mybir.dt.float32, tag="p1")
                    for kd in range(KD):
                        nc.tensor.matmul(
                            pt[:], w1t[:, kd, bass.ts(mf, 128)], xt[:, kd],
                            start=(kd == 0), stop=(kd == KD - 1))
                    nc.scalar.activation(
                        out=ht[:, mf], in_=pt[:],
                        func=mybir.ActivationFunctionType.Silu,
                        bias=b1t[:, mf], scale=1.0)
                ot = opool.tile([128, MD, CH], mybir.dt.float32, tag="o")
                for md in range(MD):
                    pt2 = ps.tile([128, CH], mybir.dt.float32, tag="p2")
                    for kf in range(KF):
                        nc.tensor.matmul(
                            pt2[:], w2t[:, kf, bass.ts(md, 128)], ht[:, kf],
                            start=(kf == 0), stop=(kf == KF - 1))
                    nc.scalar.activation(
                        out=ot[:, md], in_=pt2[:],
                        func=mybir.ActivationFunctionType.Identity,
                        bias=b2t[:, md], scale=1.0)
                    nc.sync.dma_start(out=outr[md, :, sl], in_=ot[:, md])

    from concourse.tile_patch_shim import split_multi_waits
    split_multi_waits(nc)
    return nc


def _install_walrus_shim():
    """The staged walrus allows only ONE sync-wait per instruction; install
    a concourse submodule with the drain split + wait-splitting passes."""
    import sys
    import types
    import concourse
    import concourse.tile as tile
    from concourse import mybir
    import bass_rust
    from concourse.vector_clock import ScopedClock, VectorClock

    if hasattr(concourse, "tile_patch_shim"):
        return

    def _split_drain_and_barrier(self, tick_clock, wait_clock):
        gc = tick_clock.global_clock
        n = len(gc)
        nonzero = [i for i in range(n) if gc[i] > 0]
        for i in nonzero:
            vec = [0] * n
            vec[i] = gc[i]
            d = self.nc.sync.drain()
            wait_clock.add_sem_waits(d.ins, ScopedClock({None: VectorClock(vec)}))
        if not nonzero:
            self.nc.sync.drain()
        self.nc.all_engine_barrier()
        assert self.sems is not None
        popped = self.nc._tile_sem_poison_stack.pop()
        assert popped is self._sem_poison
        self.nc.clear_and_free_semaphores(list(self.sems.allocated().values()))
        self.nc.all_engine_barrier()

    tile.TileContext._drain_and_barrier = _split_drain_and_barrier

    counter = [0]

    def split_multi_waits(nc):
        n_split = 0
        for f in nc.m.functions:
            for bb in f.blocks:
                insts = bb.instructions
                if not any(
                    i.sync_info is not None and len(i.sync_info.on_wait) > 1
                    for i in insts
                ):
                    continue
                new_list = []
                for inst in insts:
                    si = inst.sync_info
                    if si is not None and len(si.on_wait) > 1:
                        waits = list(si.on_wait)
                        for w in waits[:-1]:
                            counter[0] += 1
                            nop = mybir.InstNoOp(
                                name=f"waitnop-{counter[0]}", ins=[], outs=[])
                            nop.engine = inst.engine
                            nop.sync_info = bass_rust.SyncInfo(
                                on_wait=[w], on_update=[])
                            new_list.append(nop)
                        inst.sync_info = bass_rust.SyncInfo(
                            on_wait=[waits[-1]], on_update=list(si.on_update))
                        n_split += 1
                    new_list.append(inst)
                bb.instructions = new_list
        return n_split

    mod = types.ModuleType("concourse.tile_patch_shim")
    mod.split_multi_waits = split_multi_waits
    sys.modules["concourse.tile_patch_shim"] = mod
    concourse.tile_patch_shim = mod


def _ffn_device(xln, w1, b1, w2, b2):
    """xln: (T*B, D) LayerNormed tokens -> silu(xln@w1.T+b1)@w2.T+b2 on 8
    NeuronCores, data-parallel over tokens."""
    from concourse.bass_utils import run_bass_kernel_spmd

    if "nc" not in _FFN_CACHE:
        _install_walrus_shim()
        _FFN_CACHE["nc"] = _build_ffn_bass()
    nc = _FFN_CACHE["nc"]

    NTOK = (T * B) // N_CORES
    w1T = np.ascontiguousarray(w1.T, dtype=np.float32)  # (D, F)
    w2T = np.ascontiguousarray(w2.T, dtype=np.float32)  # (F, D)
    b1c = np.ascontiguousarray(b1.reshape(F, 1), dtype=np.float32)
    b2c = np.ascontiguousarray(b2.reshape(D, 1), dtype=np.float32)
    in_maps = []
    for c in range(N_CORES):
        shard = xln[c * NTOK : (c + 1) * NTOK]  # (NTOK, D)
        in_maps.append({
            "xT": np.ascontiguousarray(shard.T, dtype=np.float32),
            "w1T": w1T, "b1c": b1c, "w2T": w2T, "b2c": b2c,
        })
    res = run_bass_kernel_spmd(nc, in_maps, list(range(N_CORES)))
    out = np.empty((T * B, D), dtype=np.float32)
    for c in range(N_CORES):
        out[c * NTOK : (c + 1) * NTOK] = res.results[c]["outT"].T
    return out


def _ffn(x, g, b, w1, b1, w2, b2, use_device):
    xln = _ln(x, g, b).reshape(T * B, D).astype(np.float32)
    if use_device:
        try:
            h = _ffn_device(xln, w1, b1, w2, b2)
            # Spot-check a few tokens against host math; reject silently
            # wrong device output (layout bugs) rather than corrupting the
            # layer result.
            idx = np.linspace(0, T * B - 1, 32).astype(np.int64)
            ref = _silu(xln[idx] @ w1.T + b1) @ w2.T + b2
            rerr = np.linalg.norm(h[idx] - ref) / (np.linalg.norm(ref) + 1e-30)
            if not np.isfinite(rerr) or rerr > 1e-3:
                raise RuntimeError(f"device FFN self-check failed ({rerr=})")
            return h.reshape(T, B, D)
        except Exception as e:  # pragma: no cover - fallback path
            import traceback
            traceback.print_exc()
            print(f"[kernel] device FFN failed ({e!r}); host fallback")
    h = _silu(xln @ w1.T + b1)
    return (h @ w2.T + b2).reshape(T, B, D)


# ----------------------------------------------------------------- kernel


def kernel(input, key_padding_mask, dropout_prob,
           ffn1_ln_g, ffn1_ln_b, ffn1_w1, ffn1_b1, ffn1_w2, ffn1_b2,
           attn_ln_g, attn_ln_b, wq, bq, wk, bk, wv, bv, wo, bo, rel_k, rel_v,
           conv_ln_g, conv_ln_b, pw1_w, pw1_b, dw_w, dw_b, gn_g, gn_b,
           pw2_w, pw2_b,
           ffn2_ln_g, ffn2_ln_b, ffn2_w1, ffn2_b1, ffn2_w2, ffn2_b2,
           final_ln_g, final_ln_b):
    x0 = np.asarray(input, dtype=np.float32)

    use_device = True
    x = 0.5 * _ffn(x0, np.asarray(ffn1_ln_g, np.float32),
                   np.asarray(ffn1_ln_b, np.float32),
                   np.asarray(ffn1_w1, np.float32),
                   np.asarray(ffn1_b1, np.float32),
                   np.asarray(ffn1_w2, np.float32),
                   np.asarray(ffn1_b2, np.float32), use_device) + x0
    res = x
    x = _attention(_ln(x, np.asarray(attn_ln_g, np.float32),
                       np.asarray(attn_ln_b, np.float32)),
                   np.asarray(wq, np.float32), np.asarray(bq, np.float32),
                   np.asarray(wk, np.float32), np.asarray(bk, np.float32),
                   np.asarray(wv, np.float32), np.asarray(bv, np.float32),
                   np.asarray(wo, np.float32), np.asarray(bo, np.float32),
                   np.asarray(rel_k, np.float32),
                   np.asarray(rel_v, np.float32)) + res
    x = _conv_module(x, np.asarray(conv_ln_g, np.float32),
                     np.asarray(conv_ln_b, np.float32),
                     np.asarray(pw1_w, np.float32),
                     np.asarray(pw1_b, np.float32),
                     np.asarray(dw_w, np.float32),
                     np.asarray(dw_b, np.float32),
                     np.asarray(gn_g, np.float32),
                     np.asarray(gn_b, np.float32),
                     np.asarray(pw2_w, np.float32),
                     np.asarray(pw2_b, np.float32)) + x
    x = 0.5 * _ffn(x, np.asarray(ffn2_ln_g, np.float32),
                   np.asarray(ffn2_ln_b, np.float32),
                   np.asarray(ffn2_w1, np.float32),
                   np.asarray(ffn2_b1, np.float32),
                   np.asarray(ffn2_w2, np.float32),
                   np.asarray(ffn2_b2, np.float32), use_device) + x
    return _ln(x, np.asarray(final_ln_g, np.float32),
               np.asarray(final_ln_b, np.float32)).astype(np.float32)

